# revision 31
# baseline (speedup 1.0000x reference)
"""Sliding-window gated attention on 8 TRN2 NeuronCores.

Sharding: data/sequence parallel, no collectives. 2 batches x 4096 tokens
= 8192 tokens -> 8 shards of 1024 owned tokens (core c: batch c//4,
segment c%4). Each shard also receives a 256-token halo of x on the left
(the sliding window W=256 only ever reaches one block back), so every
core computes its outputs fully locally. For segment-0 cores the halo is
dummy data that the attention mask zeroes out.

Per-core layout is feature-major ("transposed"): xT [1024 dim, 1280 tok].
  rs      = 1/||x_t||           (ones-vector matmul over squared chunks)
  xhatT   = xT * rs             (row-broadcast via gpsimd partition_broadcast)
  qT,kT   = W^T @ xhatT         [feat, tok]   (fp32r matmuls)
  v       = xhatT^T @ Wv        [tok, feat]   (+ interleaved ones columns)
  scoresT = kT_h^T @ qT_h       [kpos, q]  per (head, 128-token key chunk)
  eT      = exp(scoresT) * mask{0,1}       (no max subtraction; scores are O(1))
  AV      = [v_h | 1]^T @ eT    [65, 256]: rows 0-63 unnormalized out,
                                row 64 = softmax denominator
  attgT   = AV[0:64] * (sigmoid(gate)/denom)  broadcast along partitions
  yT      = W_out^T @ attgT     [dim, tok]
RMS-norm gamma*sqrt(1024), the 1/sqrt(64) attention scale, and gamma for
the gate projection are folded into the weights host-side. All heavy
matmuls run in fp32r (fp32 with 12-bit mantissa rounding, full PE rate);
attention weights/values use bf16.
"""
import numpy as np
import ml_dtypes

import concourse.bass as bass
import concourse.tile as tile
from concourse import bacc, mybir
from concourse.bass_utils import run_bass_kernel_spmd

F32 = mybir.dt.float32
F32R = mybir.dt.float32r
BF16 = mybir.dt.bfloat16
AF = mybir.ActivationFunctionType

P = 128
DIM = 1024
HEADS = 16
DH = 64
WIN = 256
OWN = 1024          # owned tokens per core
HALO = 256
SL = OWN + HALO     # local tokens (1280)
KK = DIM // P       # 8 contraction chunks
FT = HEADS // 2     # 8 feature tiles (2 heads each)
TCH = SL // P       # 10 local token chunks
NB = OWN // WIN     # 4 owned blocks
NCORES = 8

# q-span (in owned-token coords) of each global key chunk g, and width
_G_SPAN = [(0, 256), (0, 256), (0, 512), (0, 512), (256, 512), (256, 512),
           (512, 512), (512, 512), (768, 256), (768, 256)]
# column offset of chunk g's mask inside the [128, 2048] mask tensor
_G_MASK = [1024, 1280, 0, 0, 0, 0, 0, 0, 1536, 1792]
for _g in (3, 5, 7):
    _G_MASK[_g] = 512
# statically-valid column range of each g's eT tile (outside: mask is 0,
# so exp is skipped there and the mask multiply writes the zeros)
_G_VALID = [(0, 128), (0, 256), (0, 384), (128, 384), (0, 384), (128, 384),
            (0, 384), (128, 384), (0, 256), (128, 128)]


def _round_f32r(a):
    u = np.ascontiguousarray(a, dtype=np.float32).view(np.uint32)
    r = ((u.astype(np.uint64) + 0x800) & 0xFFFFF000).astype(np.uint32)
    return r.view(np.float32).reshape(a.shape)


def _band(c):
    """{0,1} validity for key-chunk-position kp vs in-block query ql."""
    kp = np.arange(P)[:, None]
    ql = np.arange(WIN)[None, :]
    diff = 256 + ql - 128 * c - kp
    return ((diff >= 0) & (diff <= WIN)).astype(np.float32)


def _masks(first_segment):
    m_even = np.concatenate([_band(2), _band(0)], axis=1)
    m_odd = np.concatenate([_band(3), _band(1)], axis=1)
    zeros = np.zeros_like(_band(0))
    g0 = zeros if first_segment else _band(0)
    g1 = zeros if first_segment else _band(1)
    m = np.concatenate([m_even, m_odd, g0, g1, _band(2), _band(3)], axis=1)
    return m.astype(ml_dtypes.bfloat16)


def build():
    nc = bacc.Bacc("TRN2", target_bir_lowering=False, debug=False,
                   num_devices=NCORES)
    xT_d = nc.dram_tensor("xT", [DIM, SL], F32, kind="ExternalInput")
    wq_d = nc.dram_tensor("Wq", [DIM, DIM], F32R, kind="ExternalInput")
    wk_d = nc.dram_tensor("Wk", [DIM, DIM], F32R, kind="ExternalInput")
    wv_d = nc.dram_tensor("Wv", [DIM, DIM], F32R, kind="ExternalInput")
    wg_d = nc.dram_tensor("Wg", [DIM, HEADS], F32R, kind="ExternalInput")
    bg_d = nc.dram_tensor("bg", [HEADS], F32, kind="ExternalInput")
    wo_d = nc.dram_tensor("Wo", [DIM, DIM], BF16, kind="ExternalInput")
    mask_d = nc.dram_tensor("mask", [P, 2048], BF16, kind="ExternalInput")
    out_d = nc.dram_tensor("out", [DIM, OWN], F32, kind="ExternalOutput")

    lsegs = [(0, 512), (512, 512), (1024, 256)]   # local-token segments
    osegs = [(0, 512), (512, 512)]                # owned-token segments

    with tile.TileContext(nc) as tc:
        ps = tc.alloc_tile_pool(name="ps", bufs=8, space="PSUM")

        def psum(shape):
            return ps.tile(shape, F32, tag="ps", name="pst")

        def psum_acc(shape):
            return ps.tile(shape, F32, tag="ps", name="pacc")

        const_p = tc.alloc_tile_pool(name="const", bufs=1, side="left")
        mask_sb = const_p.tile([P, 2048], BF16, bufs=1)
        nc.sync.dma_start(mask_sb[:], mask_d[:])
        ones_f = const_p.tile([P, 1], F32, bufs=1)
        nc.vector.memset(ones_f[:], 1.0)
        ones_sb = const_p.tile([P, 1], F32R, bufs=1)
        nc.vector.tensor_copy(ones_sb[:], ones_f[:])
        onesr_f = const_p.tile([1, DH], F32, bufs=1)
        nc.vector.memset(onesr_f[:], 1.0)
        onesr = const_p.tile([1, DH], F32R, bufs=1)
        nc.vector.tensor_copy(onesr[:], onesr_f[:])
        bg_sb = const_p.tile([HEADS, 1], F32, bufs=1)
        nc.sync.dma_start(bg_sb[:], bg_d[:])
        eps_sb = const_p.tile([1, 1], F32, bufs=1)
        nc.vector.memset(eps_sb[:], 1e-24)
        sgT = const_p.tile([HEADS, OWN], F32, bufs=1)

        w_p = tc.alloc_tile_pool(name="w", bufs=10, side="right")
        xh_p = tc.alloc_tile_pool(name="xh", bufs=KK, side="right")
        x_p = tc.alloc_tile_pool(name="x", bufs=KK, side="right")
        x2_p = tc.alloc_tile_pool(name="x2", bufs=2, side="right")

        def wload(dram, kk, name):
            wt = w_p.tile([P, DIM], F32R, tag="w", name=name)
            nc.sync.dma_start(wt[:], dram[kk * P:(kk + 1) * P, :])
            return wt

        wq_sb = [wload(wq_d, kk, f"wq{kk}") for kk in range(KK)]
        x_sb = []
        for kk in range(KK):
            xt = x_p.tile([P, SL], F32, tag="xT", name=f"x{kk}")
            nc.sync.dma_start(xt[:], xT_d[kk * P:(kk + 1) * P, :])
            x_sb.append(xt)

        # ---- norm: rs = 1/sqrt(sum_d x^2) ------------------------------
        rs_row = x2_p.tile([1, SL], F32, bufs=1)
        rsb = x2_p.tile([P, SL], F32, bufs=1)
        ssq_ps = [psum([1, w]) for _, w in lsegs]
        for kk in range(KK):
            x2 = x2_p.tile([P, SL], F32R, tag="x2", name=f"x2_{kk}")
            nc.scalar.activation(x2[:], x_sb[kk][:], AF.Square)
            for si, (s0, w) in enumerate(lsegs):
                nc.tensor.matmul(ssq_ps[si][:], ones_sb[:], x2[:, s0:s0 + w],
                                 start=(kk == 0), stop=(kk == KK - 1))
        for si, (s0, w) in enumerate(lsegs):
            nrm = x2_p.tile([1, 512], F32, tag="nrm", name=f"nrm{si}")
            nc.scalar.activation(nrm[:1, :w], ssq_ps[si][:], AF.Sqrt,
                                 bias=eps_sb[:])
            nc.vector.reciprocal_approx_fast(rs_row[:, s0:s0 + w],
                                             nrm[:1, :w])
        nc.gpsimd.partition_broadcast(rsb[:], rs_row[:])

        xh_sb = []
        for kk in range(KK):
            xh = xh_p.tile([P, SL], F32R, tag="xh", name=f"xh{kk}")
            nc.vector.tensor_mul(xh[:], x_sb[kk][:], rsb[:])
            xh_sb.append(xh)
        x2_p.release()
        x_p.release()

        # ---- projections ----------------------------------------------
        q_p = tc.alloc_tile_pool(name="q", bufs=FT, side="left")
        k_p = tc.alloc_tile_pool(name="k", bufs=FT, side="left")
        v_p = tc.alloc_tile_pool(name="v", bufs=TCH, side="left")
        sgal_p = tc.alloc_tile_pool(name="sgal", bufs=4, side="left")
        wg_p = tc.alloc_tile_pool(name="wg", bufs=KK, side="right")
        # gate rows spread to aligned partitions (0/32/64/96), 4 heads/tile
        sg_al = [sgal_p.tile([P, OWN], F32, tag="sgal", name=f"sga{t}", bufs=1)
                 for t in range(4)]

        # Q pass (owned tokens only)
        qT = [q_p.tile([P, OWN], F32R, tag="qT", name=f"qT{ft}")
              for ft in range(FT)]
        for ft in range(FT):
            for s0, w in osegs:
                acc = psum([P, w])
                for kk in range(KK):
                    nc.tensor.matmul(
                        acc[:], wq_sb[kk][:, ft * P:(ft + 1) * P],
                        xh_sb[kk][:, HALO + s0:HALO + s0 + w],
                        start=(kk == 0), stop=(kk == KK - 1))
                nc.scalar.copy(qT[ft][:, s0:s0 + w], acc[:])

        # K pass (all local tokens)
        wk_sb = [wload(wk_d, kk, f"wk{kk}") for kk in range(KK)]
        kT = [k_p.tile([P, SL], F32R, tag="kT", name=f"kT{ft}")
              for ft in range(FT)]
        for ft in range(FT):
            for s0, w in lsegs:
                acc = psum([P, w])
                for kk in range(KK):
                    nc.tensor.matmul(
                        acc[:], wk_sb[kk][:, ft * P:(ft + 1) * P],
                        xh_sb[kk][:, s0:s0 + w],
                        start=(kk == 0), stop=(kk == KK - 1))
                nc.scalar.copy(kT[ft][:, s0:s0 + w], acc[:])

        # V pass -> token-major with interleaved ones columns
        wv_sb = [wload(wv_d, kk, f"wv{kk}") for kk in range(KK)]
        v_sb = []
        for g in range(TCH):
            vt = v_p.tile([P, HEADS * (DH + 1)], BF16, tag="v", name=f"v{g}")
            v3 = vt.rearrange("p (h e) -> p h e", e=DH + 1)
            nc.vector.memset(v3[:, :, DH:DH + 1], 1.0)
            for fh in range(2):
                acc = psum([P, 512])
                for kk in range(KK):
                    nc.tensor.matmul(
                        acc[:], xh_sb[kk][:, g * P:(g + 1) * P],
                        wv_sb[kk][:, fh * 512:(fh + 1) * 512],
                        start=(kk == 0), stop=(kk == KK - 1))
                nc.vector.tensor_copy(v3[:, 8 * fh:8 * (fh + 1), 0:DH], acc[:])
            v_sb.append(v3)

        # gates -> sigmoid(x @ Wg + bg), head-major [16, 1024]
        wg_sb = []
        for kk in range(KK):
            wgt = wg_p.tile([P, HEADS], F32R, tag="wg", name=f"wgk{kk}")
            nc.sync.dma_start(wgt[:], wg_d[kk * P:(kk + 1) * P, :])
            wg_sb.append(wgt)
        for s0, w in osegs:
            acc = psum([HEADS, w])
            for kk in range(KK):
                nc.tensor.matmul(acc[:], wg_sb[kk][:],
                                 xh_sb[kk][:, HALO + s0:HALO + s0 + w],
                                 start=(kk == 0), stop=(kk == KK - 1))
            nc.scalar.activation(sgT[:, s0:s0 + w], acc[:], AF.Sigmoid,
                                 bias=bg_sb[:])
        for t in range(4):
            dst = sg_al[t].rearrange("(a b) n -> a b n", b=32)[:, 0:1, :]
            nc.sync.dma_start(dst, sgT[4 * t:4 * (t + 1), :])
        wg_p.release()
        xh_p.release()
        w_p.release()

        # ---- attention --------------------------------------------------
        ag_p = tc.alloc_tile_pool(name="ag", bufs=FT, side="right")
        wo_p = tc.alloc_tile_pool(name="wo", bufs=KK, side="right")
        e_p = tc.alloc_tile_pool(name="e", bufs=32, side="right")
        av_p = tc.alloc_tile_pool(name="av", bufs=2, side="right")
        wo_sb = []
        for t in range(KK):
            wot = wo_p.tile([P, DIM], BF16, tag="wo", name=f"wo{t}")
            nc.sync.dma_start(wot[:], wo_d[t * P:(t + 1) * P, :])
            wo_sb.append(wot)
        agT = [ag_p.tile([P, OWN], BF16, tag="agT", name=f"agT{ft}")
               for ft in range(FT)]
        def emit_scores(ft, h2):
            h = 2 * ft + h2
            hp = h2 * DH
            # gate row for head h staged at partition 0 (HW
            # partition_broadcast always reads physical partition 0)
            pa = 32 * (h % 4)
            sg0 = av_p.tile([1, OWN], F32, tag="sg0", name=f"sg0_{h}",
                            bufs=4)
            nc.vector.tensor_copy(sg0[:], sg_al[h // 4][pa:pa + 1, :])
            eT = [None] * TCH
            for g in (2, 3, 0, 1, 6, 7, 4, 5, 8, 9):
                qs, w = _G_SPAN[g]
                v0, vw = _G_VALID[g]
                sc = psum([P, vw])
                nc.tensor.matmul(
                    sc[:], kT[ft][hp:hp + DH, g * P:(g + 1) * P],
                    qT[ft][hp:hp + DH, qs + v0:qs + v0 + vw],
                    start=True, stop=True)
                e = e_p.tile([P, 512], BF16, tag="eT", name=f"e{g}")
                nc.scalar.activation(e[:, v0:v0 + vw], sc[:], AF.Exp)
                mc = _G_MASK[g]
                nc.vector.tensor_mul(e[:, v0:v0 + vw], e[:, v0:v0 + vw],
                                     mask_sb[:, mc + v0:mc + v0 + vw])
                if v0 > 0:
                    nc.vector.memset(e[:, 0:v0], 0.0)
                if v0 + vw < w:
                    nc.vector.memset(e[:, v0 + vw:w], 0.0)
                eT[g] = e
            return sg0, eT

        def emit_av(ft, h2, sg0, eT):
            h = 2 * ft + h2
            hp = h2 * DH
            for i in range(2):
                # block pair (2i, 2i+1): one [65, 512] accumulation.
                # full-width matmuls first so start=True overwrites the
                # whole region before partial-width accumulates land.
                acc = psum_acc([DH + 1, 2 * WIN])
                base = 4 * i
                parts = [(base + 2, 0, 0, 512), (base + 3, 0, 0, 512),
                         (base + 0, 2 * i * WIN - _G_SPAN[base][0], 0, WIN),
                         (base + 1, 2 * i * WIN - _G_SPAN[base + 1][0],
                          0, WIN),
                         (base + 4,
                          (2 * i + 1) * WIN - _G_SPAN[base + 4][0],
                          WIN, WIN),
                         (base + 5,
                          (2 * i + 1) * WIN - _G_SPAN[base + 5][0],
                          WIN, WIN)]
                for j, (g, sect, p0, pw) in enumerate(parts):
                    nc.tensor.matmul(
                        acc[:, p0:p0 + pw], v_sb[g][:, h, :],
                        eT[g][:, sect:sect + pw],
                        start=(j == 0), stop=(j == len(parts) - 1),
                        skip_group_check=True)
                # scale = sigmoid(gate)/denominator; broadcast along
                # partitions via a rank-1 (ones x crow) matmul on PE
                srow = av_p.tile([1, 2 * WIN], F32, tag="srow",
                                 name=f"sr{i}")
                nc.vector.tensor_copy(srow[:], acc[DH:DH + 1, :])
                sinv = av_p.tile([1, 2 * WIN], F32, tag="sinv",
                                 name=f"si{i}")
                nc.vector.reciprocal_approx_fast(sinv[:], srow[:])
                crow = av_p.tile([1, 2 * WIN], F32, tag="crow",
                                 name=f"cr{i}")
                nc.vector.tensor_mul(
                    crow[:], sinv[:],
                    sg0[:, 2 * i * WIN:2 * (i + 1) * WIN])
                cb = av_p.tile([DH, 2 * WIN], F32, tag="cb",
                               name=f"cb{i}")
                nc.gpsimd.partition_broadcast(cb[:], crow[:])
                nc.vector.tensor_mul(
                    agT[ft][hp:hp + DH, 2 * i * WIN:2 * (i + 1) * WIN],
                    acc[0:DH, :], cb[:])


        from collections import deque
        pend = deque()
        for ft in range(FT):
            for h2 in range(2):
                pend.append((ft, h2, *emit_scores(ft, h2)))
                if len(pend) > 1:
                    emit_av(*pend.popleft())
        while pend:
            emit_av(*pend.popleft())

        # ---- output projection -----------------------------------------
        av_p.release()
        e_p.release()
        sgal_p.release()
        v_p.release()
        k_p.release()
        q_p.release()
        y_p = tc.alloc_tile_pool(name="y", bufs=3, side="right")
        for dt in range(KK):
            yt = y_p.tile([P, OWN], F32, tag="yt", name=f"yt{dt}")
            for s0, w in osegs:
                acc = psum([P, w])
                for t in range(KK):
                    nc.tensor.matmul(acc[:], wo_sb[t][:, dt * P:(dt + 1) * P],
                                     agT[t][:, s0:s0 + w],
                                     start=(t == 0), stop=(t == KK - 1))
                nc.scalar.copy(yt[:, s0:s0 + w], acc[:])
            nc.sync.dma_start(out_d[dt * P:(dt + 1) * P, :], yt[:])

        y_p.release()
        wo_p.release()
        ag_p.release()
        const_p.release()
        ps.release()

    nc.compile()
    return nc


def make_in_maps(x, gamma, W_qkv, W_gates, b_gates, W_out):
    b, S, dim = x.shape
    assert (b, S, dim) == (2, 4096, DIM)
    g32 = (gamma * (dim ** 0.5)).astype(np.float32)
    wqkv = W_qkv * g32[:, None]
    wq = _round_f32r(wqkv[:, :DIM] * (DH ** -0.5))
    wk = _round_f32r(wqkv[:, DIM:2 * DIM])
    wv = _round_f32r(wqkv[:, 2 * DIM:3 * DIM])
    wg = _round_f32r(W_gates * g32[:, None])
    wo = np.asarray(W_out, np.float32).astype(ml_dtypes.bfloat16)
    bg = np.ascontiguousarray(b_gates, dtype=np.float32)
    m_first = _masks(True)
    m_rest = _masks(False)

    in_maps = []
    for c in range(NCORES):
        bb, seg = c // 4, c % 4
        own = x[bb, seg * OWN:(seg + 1) * OWN]
        halo = x[bb, seg * OWN - HALO: seg * OWN] if seg else x[bb, :HALO]
        xT = np.ascontiguousarray(
            np.concatenate([halo, own], axis=0).T, dtype=np.float32)
        in_maps.append({
            "xT": xT, "Wq": wq, "Wk": wk, "Wv": wv, "Wg": wg, "bg": bg,
            "Wo": wo, "mask": m_first if seg == 0 else m_rest,
        })
    return in_maps


_NC_CACHE = []


def kernel(x, gamma, W_qkv, W_gates, b_gates, W_out):
    x = np.asarray(x, dtype=np.float32)
    in_maps = make_in_maps(
        x, np.asarray(gamma, np.float32), np.asarray(W_qkv, np.float32),
        np.asarray(W_gates, np.float32), np.asarray(b_gates, np.float32),
        np.asarray(W_out, np.float32))
    if not _NC_CACHE:
        _NC_CACHE.append(build())
    nc = _NC_CACHE[0]
    res = run_bass_kernel_spmd(nc, in_maps, core_ids=list(range(NCORES)))
    y = np.empty((2, 4096, DIM), dtype=np.float32)
    for c in range(NCORES):
        bb, seg = c // 4, c % 4
        y[bb, seg * OWN:(seg + 1) * OWN] = res.results[c]["out"].T
    return y


# revision 32
# speedup vs baseline: 1.0084x; 1.0084x over previous
"""Sliding-window gated attention on 8 TRN2 NeuronCores.

Sharding: data/sequence parallel, no collectives. 2 batches x 4096 tokens
= 8192 tokens -> 8 shards of 1024 owned tokens (core c: batch c//4,
segment c%4). Each shard also receives a 256-token halo of x on the left
(the sliding window W=256 only ever reaches one block back), so every
core computes its outputs fully locally. For segment-0 cores the halo is
dummy data that the attention mask zeroes out.

Per-core layout is feature-major ("transposed"): xT [1024 dim, 1280 tok].
  rs      = 1/||x_t||           (ones-vector matmul over squared chunks)
  xhatT   = xT * rs             (row-broadcast via gpsimd partition_broadcast)
  qT,kT   = W^T @ xhatT         [feat, tok]   (fp32r matmuls)
  v       = xhatT^T @ Wv        [tok, feat]   (+ interleaved ones columns)
  scoresT = kT_h^T @ qT_h       [kpos, q]  per (head, 128-token key chunk)
  eT      = exp(scoresT) * mask{0,1}       (no max subtraction; scores are O(1))
  AV      = [v_h | 1]^T @ eT    [65, 256]: rows 0-63 unnormalized out,
                                row 64 = softmax denominator
  attgT   = AV[0:64] * (sigmoid(gate)/denom)  broadcast along partitions
  yT      = W_out^T @ attgT     [dim, tok]
RMS-norm gamma*sqrt(1024), the 1/sqrt(64) attention scale, and gamma for
the gate projection are folded into the weights host-side. All heavy
matmuls run in fp32r (fp32 with 12-bit mantissa rounding, full PE rate);
attention weights/values use bf16.
"""
import numpy as np
import ml_dtypes

import concourse.bass as bass
import concourse.tile as tile
from concourse import bacc, mybir
from concourse.bass_utils import run_bass_kernel_spmd

F32 = mybir.dt.float32
F32R = mybir.dt.float32r
BF16 = mybir.dt.bfloat16
AF = mybir.ActivationFunctionType

P = 128
DIM = 1024
HEADS = 16
DH = 64
WIN = 256
OWN = 1024          # owned tokens per core
HALO = 256
SL = OWN + HALO     # local tokens (1280)
KK = DIM // P       # 8 contraction chunks
FT = HEADS // 2     # 8 feature tiles (2 heads each)
TCH = SL // P       # 10 local token chunks
NB = OWN // WIN     # 4 owned blocks
NCORES = 8

# q-span (in owned-token coords) of each global key chunk g, and width
_G_SPAN = [(0, 256), (0, 256), (0, 512), (0, 512), (256, 512), (256, 512),
           (512, 512), (512, 512), (768, 256), (768, 256)]
# column offset of chunk g's mask inside the [128, 2048] mask tensor
_G_MASK = [1024, 1280, 0, 0, 0, 0, 0, 0, 1536, 1792]
for _g in (3, 5, 7):
    _G_MASK[_g] = 512
# statically-valid column range of each g's eT tile (outside: mask is 0,
# so exp is skipped there and the mask multiply writes the zeros)
_G_VALID = [(0, 128), (0, 256), (0, 384), (128, 384), (0, 384), (128, 384),
            (0, 384), (128, 384), (0, 256), (128, 128)]


def _round_f32r(a):
    u = np.ascontiguousarray(a, dtype=np.float32).view(np.uint32)
    r = ((u.astype(np.uint64) + 0x800) & 0xFFFFF000).astype(np.uint32)
    return r.view(np.float32).reshape(a.shape)


def _band(c):
    """{0,1} validity for key-chunk-position kp vs in-block query ql."""
    kp = np.arange(P)[:, None]
    ql = np.arange(WIN)[None, :]
    diff = 256 + ql - 128 * c - kp
    return ((diff >= 0) & (diff <= WIN)).astype(np.float32)


def _masks(first_segment):
    m_even = np.concatenate([_band(2), _band(0)], axis=1)
    m_odd = np.concatenate([_band(3), _band(1)], axis=1)
    zeros = np.zeros_like(_band(0))
    g0 = zeros if first_segment else _band(0)
    g1 = zeros if first_segment else _band(1)
    m = np.concatenate([m_even, m_odd, g0, g1, _band(2), _band(3)], axis=1)
    return m.astype(ml_dtypes.bfloat16)


def build():
    nc = bacc.Bacc("TRN2", target_bir_lowering=False, debug=False,
                   num_devices=NCORES)
    xT_d = nc.dram_tensor("xT", [DIM, SL], F32, kind="ExternalInput")
    wq_d = nc.dram_tensor("Wq", [DIM, DIM], F32R, kind="ExternalInput")
    wk_d = nc.dram_tensor("Wk", [DIM, DIM], F32R, kind="ExternalInput")
    wv_d = nc.dram_tensor("Wv", [DIM, DIM], F32R, kind="ExternalInput")
    wg_d = nc.dram_tensor("Wg", [DIM, HEADS], F32R, kind="ExternalInput")
    bg_d = nc.dram_tensor("bg", [HEADS], F32, kind="ExternalInput")
    wo_d = nc.dram_tensor("Wo", [DIM, DIM], BF16, kind="ExternalInput")
    mask_d = nc.dram_tensor("mask", [P, 2048], BF16, kind="ExternalInput")
    out_d = nc.dram_tensor("out", [DIM, OWN], F32, kind="ExternalOutput")

    lsegs = [(0, 512), (512, 512), (1024, 256)]   # local-token segments
    osegs = [(0, 512), (512, 512)]                # owned-token segments

    with tile.TileContext(nc) as tc:
        ps = tc.alloc_tile_pool(name="ps", bufs=8, space="PSUM")

        def psum(shape):
            return ps.tile(shape, F32, tag="ps", name="pst")

        def psum_acc(shape):
            return ps.tile(shape, F32, tag="ps", name="pacc")

        const_p = tc.alloc_tile_pool(name="const", bufs=1, side="left")
        mask_sb = const_p.tile([P, 2048], BF16, bufs=1)
        nc.sync.dma_start(mask_sb[:], mask_d[:])
        ones_f = const_p.tile([P, 1], F32, bufs=1)
        nc.vector.memset(ones_f[:], 1.0)
        ones_sb = const_p.tile([P, 1], F32R, bufs=1)
        nc.vector.tensor_copy(ones_sb[:], ones_f[:])
        onesr_f = const_p.tile([1, DH], F32, bufs=1)
        nc.vector.memset(onesr_f[:], 1.0)
        onesr = const_p.tile([1, DH], F32R, bufs=1)
        nc.vector.tensor_copy(onesr[:], onesr_f[:])
        bg_sb = const_p.tile([HEADS, 1], F32, bufs=1)
        nc.sync.dma_start(bg_sb[:], bg_d[:])
        eps_sb = const_p.tile([1, 1], F32, bufs=1)
        nc.vector.memset(eps_sb[:], 1e-24)
        sgT = const_p.tile([HEADS, OWN], F32, bufs=1)

        w_p = tc.alloc_tile_pool(name="w", bufs=10, side="right")
        xh_p = tc.alloc_tile_pool(name="xh", bufs=KK, side="right")
        x_p = tc.alloc_tile_pool(name="x", bufs=KK, side="right")
        x2_p = tc.alloc_tile_pool(name="x2", bufs=2, side="right")

        def wload(dram, kk, name):
            wt = w_p.tile([P, DIM], F32R, tag="w", name=name)
            nc.sync.dma_start(wt[:], dram[kk * P:(kk + 1) * P, :])
            return wt

        wq_sb = [wload(wq_d, kk, f"wq{kk}") for kk in range(KK)]
        x_sb = []
        for kk in range(KK):
            xt = x_p.tile([P, SL], F32, tag="xT", name=f"x{kk}")
            nc.sync.dma_start(xt[:], xT_d[kk * P:(kk + 1) * P, :])
            x_sb.append(xt)

        # ---- norm: rs = 1/sqrt(sum_d x^2) ------------------------------
        rs_row = x2_p.tile([1, SL], F32, bufs=1)
        rsb = x2_p.tile([P, SL], F32, bufs=1)
        ssq_ps = [psum([1, w]) for _, w in lsegs]
        for kk in range(KK):
            x2 = x2_p.tile([P, SL], F32R, tag="x2", name=f"x2_{kk}")
            nc.scalar.activation(x2[:], x_sb[kk][:], AF.Square)
            for si, (s0, w) in enumerate(lsegs):
                nc.tensor.matmul(ssq_ps[si][:], ones_sb[:], x2[:, s0:s0 + w],
                                 start=(kk == 0), stop=(kk == KK - 1))
        for si, (s0, w) in enumerate(lsegs):
            nrm = x2_p.tile([1, 512], F32, tag="nrm", name=f"nrm{si}")
            nc.scalar.activation(nrm[:1, :w], ssq_ps[si][:], AF.Sqrt,
                                 bias=eps_sb[:])
            nc.vector.reciprocal_approx_fast(rs_row[:, s0:s0 + w],
                                             nrm[:1, :w])
        nc.gpsimd.partition_broadcast(rsb[:], rs_row[:])

        xh_sb = []
        for kk in range(KK):
            xh = xh_p.tile([P, SL], F32R, tag="xh", name=f"xh{kk}")
            nc.vector.tensor_mul(xh[:], x_sb[kk][:], rsb[:])
            xh_sb.append(xh)
        x2_p.release()
        x_p.release()

        # ---- projections ----------------------------------------------
        q_p = tc.alloc_tile_pool(name="q", bufs=FT, side="left")
        k_p = tc.alloc_tile_pool(name="k", bufs=FT, side="left")
        v_p = tc.alloc_tile_pool(name="v", bufs=TCH, side="left")
        sgal_p = tc.alloc_tile_pool(name="sgal", bufs=4, side="left")
        wg_p = tc.alloc_tile_pool(name="wg", bufs=KK, side="right")
        # gate rows spread to aligned partitions (0/32/64/96), 4 heads/tile
        sg_al = [sgal_p.tile([P, OWN], F32, tag="sgal", name=f"sga{t}", bufs=1)
                 for t in range(4)]

        # Q pass (owned tokens only)
        qT = [q_p.tile([P, OWN], F32R, tag="qT", name=f"qT{ft}")
              for ft in range(FT)]
        for ft in range(FT):
            for s0, w in osegs:
                acc = psum([P, w])
                for kk in range(KK):
                    nc.tensor.matmul(
                        acc[:], wq_sb[kk][:, ft * P:(ft + 1) * P],
                        xh_sb[kk][:, HALO + s0:HALO + s0 + w],
                        start=(kk == 0), stop=(kk == KK - 1))
                nc.scalar.copy(qT[ft][:, s0:s0 + w], acc[:])

        # K pass (all local tokens)
        wk_sb = [wload(wk_d, kk, f"wk{kk}") for kk in range(KK)]
        kT = [k_p.tile([P, SL], F32R, tag="kT", name=f"kT{ft}")
              for ft in range(FT)]
        for ft in range(FT):
            for s0, w in lsegs:
                acc = psum([P, w])
                for kk in range(KK):
                    nc.tensor.matmul(
                        acc[:], wk_sb[kk][:, ft * P:(ft + 1) * P],
                        xh_sb[kk][:, s0:s0 + w],
                        start=(kk == 0), stop=(kk == KK - 1))
                nc.scalar.copy(kT[ft][:, s0:s0 + w], acc[:])

        # V pass -> token-major with interleaved ones columns
        wv_sb = [wload(wv_d, kk, f"wv{kk}") for kk in range(KK)]
        v_sb = []
        for g in range(TCH):
            vt = v_p.tile([P, HEADS * (DH + 1)], BF16, tag="v", name=f"v{g}")
            v3 = vt.rearrange("p (h e) -> p h e", e=DH + 1)
            nc.vector.memset(v3[:, :, DH:DH + 1], 1.0)
            for fh in range(2):
                acc = psum([P, 512])
                for kk in range(KK):
                    nc.tensor.matmul(
                        acc[:], xh_sb[kk][:, g * P:(g + 1) * P],
                        wv_sb[kk][:, fh * 512:(fh + 1) * 512],
                        start=(kk == 0), stop=(kk == KK - 1))
                nc.vector.tensor_copy(v3[:, 8 * fh:8 * (fh + 1), 0:DH], acc[:])
            v_sb.append(v3)

        # gates -> sigmoid(x @ Wg + bg), head-major [16, 1024]
        wg_sb = []
        for kk in range(KK):
            wgt = wg_p.tile([P, HEADS], F32R, tag="wg", name=f"wgk{kk}")
            nc.sync.dma_start(wgt[:], wg_d[kk * P:(kk + 1) * P, :])
            wg_sb.append(wgt)
        for s0, w in osegs:
            acc = psum([HEADS, w])
            for kk in range(KK):
                nc.tensor.matmul(acc[:], wg_sb[kk][:],
                                 xh_sb[kk][:, HALO + s0:HALO + s0 + w],
                                 start=(kk == 0), stop=(kk == KK - 1))
            nc.scalar.activation(sgT[:, s0:s0 + w], acc[:], AF.Sigmoid,
                                 bias=bg_sb[:])
        for t in range(4):
            dst = sg_al[t].rearrange("(a b) n -> a b n", b=32)[:, 0:1, :]
            nc.sync.dma_start(dst, sgT[4 * t:4 * (t + 1), :])
        wg_p.release()
        xh_p.release()
        w_p.release()

        # ---- attention --------------------------------------------------
        ag_p = tc.alloc_tile_pool(name="ag", bufs=FT, side="right")
        wo_p = tc.alloc_tile_pool(name="wo", bufs=KK, side="right")
        e_p = tc.alloc_tile_pool(name="e", bufs=32, side="right")
        av_p = tc.alloc_tile_pool(name="av", bufs=2, side="right")
        wo_sb = []
        for t in range(KK):
            wot = wo_p.tile([P, DIM], BF16, tag="wo", name=f"wo{t}")
            nc.sync.dma_start(wot[:], wo_d[t * P:(t + 1) * P, :])
            wo_sb.append(wot)
        agT = [ag_p.tile([P, OWN], BF16, tag="agT", name=f"agT{ft}")
               for ft in range(FT)]
        def emit_scores(ft, h2):
            h = 2 * ft + h2
            hp = h2 * DH
            # gate row for head h staged at partition 0 (HW
            # partition_broadcast always reads physical partition 0)
            pa = 32 * (h % 4)
            sg0 = av_p.tile([1, OWN], F32, tag="sg0", name=f"sg0_{h}",
                            bufs=4)
            nc.scalar.copy(sg0[:], sg_al[h // 4][pa:pa + 1, :])
            eT = [None] * TCH
            for g in (2, 3, 0, 1, 6, 7, 4, 5, 8, 9):
                qs, w = _G_SPAN[g]
                v0, vw = _G_VALID[g]
                sc = psum([P, vw])
                nc.tensor.matmul(
                    sc[:], kT[ft][hp:hp + DH, g * P:(g + 1) * P],
                    qT[ft][hp:hp + DH, qs + v0:qs + v0 + vw],
                    start=True, stop=True)
                e = e_p.tile([P, 512], BF16, tag="eT", name=f"e{g}")
                nc.scalar.activation(e[:, v0:v0 + vw], sc[:], AF.Exp)
                mc = _G_MASK[g]
                nc.vector.tensor_mul(e[:, v0:v0 + vw], e[:, v0:v0 + vw],
                                     mask_sb[:, mc + v0:mc + v0 + vw])
                if v0 > 0:
                    nc.vector.memset(e[:, 0:v0], 0.0)
                if v0 + vw < w:
                    nc.vector.memset(e[:, v0 + vw:w], 0.0)
                eT[g] = e
            return sg0, eT

        def emit_av(ft, h2, sg0, eT):
            h = 2 * ft + h2
            hp = h2 * DH
            for i in range(2):
                # block pair (2i, 2i+1): one [65, 512] accumulation.
                # full-width matmuls first so start=True overwrites the
                # whole region before partial-width accumulates land.
                acc = psum_acc([DH + 1, 2 * WIN])
                base = 4 * i
                parts = [(base + 2, 0, 0, 512), (base + 3, 0, 0, 512),
                         (base + 0, 2 * i * WIN - _G_SPAN[base][0], 0, WIN),
                         (base + 1, 2 * i * WIN - _G_SPAN[base + 1][0],
                          0, WIN),
                         (base + 4,
                          (2 * i + 1) * WIN - _G_SPAN[base + 4][0],
                          WIN, WIN),
                         (base + 5,
                          (2 * i + 1) * WIN - _G_SPAN[base + 5][0],
                          WIN, WIN)]
                for j, (g, sect, p0, pw) in enumerate(parts):
                    nc.tensor.matmul(
                        acc[:, p0:p0 + pw], v_sb[g][:, h, :],
                        eT[g][:, sect:sect + pw],
                        start=(j == 0), stop=(j == len(parts) - 1),
                        skip_group_check=True)
                # scale = sigmoid(gate)/denominator; broadcast along
                # partitions via a rank-1 (ones x crow) matmul on PE
                srow = av_p.tile([1, 2 * WIN], F32, tag="srow",
                                 name=f"sr{i}")
                nc.vector.tensor_copy(srow[:], acc[DH:DH + 1, :])
                sinv = av_p.tile([1, 2 * WIN], F32, tag="sinv",
                                 name=f"si{i}")
                nc.vector.reciprocal_approx_fast(sinv[:], srow[:])
                crow = av_p.tile([1, 2 * WIN], F32, tag="crow",
                                 name=f"cr{i}")
                nc.vector.tensor_mul(
                    crow[:], sinv[:],
                    sg0[:, 2 * i * WIN:2 * (i + 1) * WIN])
                cb = av_p.tile([DH, 2 * WIN], F32, tag="cb",
                               name=f"cb{i}")
                nc.gpsimd.partition_broadcast(cb[:], crow[:])
                nc.vector.tensor_mul(
                    agT[ft][hp:hp + DH, 2 * i * WIN:2 * (i + 1) * WIN],
                    acc[0:DH, :], cb[:])


        from collections import deque
        pend = deque()
        for ft in range(FT):
            for h2 in range(2):
                pend.append((ft, h2, *emit_scores(ft, h2)))
                if len(pend) > 1:
                    emit_av(*pend.popleft())
        while pend:
            emit_av(*pend.popleft())

        # ---- output projection -----------------------------------------
        av_p.release()
        e_p.release()
        sgal_p.release()
        v_p.release()
        k_p.release()
        q_p.release()
        y_p = tc.alloc_tile_pool(name="y", bufs=3, side="right")
        for dt in range(KK):
            yt = y_p.tile([P, OWN], F32, tag="yt", name=f"yt{dt}")
            for s0, w in osegs:
                acc = psum([P, w])
                for t in range(KK):
                    nc.tensor.matmul(acc[:], wo_sb[t][:, dt * P:(dt + 1) * P],
                                     agT[t][:, s0:s0 + w],
                                     start=(t == 0), stop=(t == KK - 1))
                nc.scalar.copy(yt[:, s0:s0 + w], acc[:])
            nc.sync.dma_start(out_d[dt * P:(dt + 1) * P, :], yt[:])

        y_p.release()
        wo_p.release()
        ag_p.release()
        const_p.release()
        ps.release()

    nc.compile()
    return nc


def make_in_maps(x, gamma, W_qkv, W_gates, b_gates, W_out):
    b, S, dim = x.shape
    assert (b, S, dim) == (2, 4096, DIM)
    g32 = (gamma * (dim ** 0.5)).astype(np.float32)
    wqkv = W_qkv * g32[:, None]
    wq = _round_f32r(wqkv[:, :DIM] * (DH ** -0.5))
    wk = _round_f32r(wqkv[:, DIM:2 * DIM])
    wv = _round_f32r(wqkv[:, 2 * DIM:3 * DIM])
    wg = _round_f32r(W_gates * g32[:, None])
    wo = np.asarray(W_out, np.float32).astype(ml_dtypes.bfloat16)
    bg = np.ascontiguousarray(b_gates, dtype=np.float32)
    m_first = _masks(True)
    m_rest = _masks(False)

    in_maps = []
    for c in range(NCORES):
        bb, seg = c // 4, c % 4
        own = x[bb, seg * OWN:(seg + 1) * OWN]
        halo = x[bb, seg * OWN - HALO: seg * OWN] if seg else x[bb, :HALO]
        xT = np.ascontiguousarray(
            np.concatenate([halo, own], axis=0).T, dtype=np.float32)
        in_maps.append({
            "xT": xT, "Wq": wq, "Wk": wk, "Wv": wv, "Wg": wg, "bg": bg,
            "Wo": wo, "mask": m_first if seg == 0 else m_rest,
        })
    return in_maps


_NC_CACHE = []


def kernel(x, gamma, W_qkv, W_gates, b_gates, W_out):
    x = np.asarray(x, dtype=np.float32)
    in_maps = make_in_maps(
        x, np.asarray(gamma, np.float32), np.asarray(W_qkv, np.float32),
        np.asarray(W_gates, np.float32), np.asarray(b_gates, np.float32),
        np.asarray(W_out, np.float32))
    if not _NC_CACHE:
        _NC_CACHE.append(build())
    nc = _NC_CACHE[0]
    res = run_bass_kernel_spmd(nc, in_maps, core_ids=list(range(NCORES)))
    y = np.empty((2, 4096, DIM), dtype=np.float32)
    for c in range(NCORES):
        bb, seg = c // 4, c % 4
        y[bb, seg * OWN:(seg + 1) * OWN] = res.results[c]["out"].T
    return y
